# revision 9
# baseline (speedup 1.0000x reference)
"""MANO/SMPL forward kernel for Trainium2, 8-core data-parallel SPMD.

Strategy (per core, batch shard of 512):
  1. Pose matmul on PE: [theta|1|wrist] @ R49x60 -> per-slot euler angles
     (20 slots: 5 wrist/root copies + 15 pose joints in level-major order).
  2. Rodrigues (wide elementwise ops, batch on partitions, slots in free dim).
  3. Kinematic chain by tree level (3 levels x 5 fingers vectorized).
  4. A_rel translation fixup.
  5. LBS folded into a single K-split matmul: verts/joints = Aflat @ RHS,
     where RHS embeds (weights x [v_template|1]) with the 3 output coords
     interleaved so PSUM holds the final HBM layout directly.
All batch-independent constants are precomputed on the host and fed as
replicated DRAM inputs.  Chain tiles use free layout (entry, slot*4+cb) so
every elementwise op needs at most 3 free AP dims.
"""
import sys
import types
import numpy as np

import concourse.bacc as bacc
import concourse.tile as tile
from concourse import mybir, masks
from concourse.bass_utils import run_bass_kernel_spmd

dt = mybir.dt
F32 = dt.float32
F32R = dt.float32r
AF = mybir.ActivationFunctionType
ALU = mybir.AluOpType

B, NV, NCOMPS = 4096, 1084, 45
NCORES = 8
BC = B // NCORES          # 512 rows per core
NCB = BC // 128           # 4 chunks of 128 partitions
NJOUT = 21
NCOLS = 3 * NV + 3 * NJOUT   # 3315 output cols per batch row
NVJ = NV + NJOUT             # 1105 columns per coordinate
NVJP = NVJ + 1               # padded even
VCHUNKS = [(0, 370), (370, 370), (740, 366)]  # all >=256 and even
N_WARM = 45

PARENTS = [-1, 0, 1, 2, 0, 4, 5, 0, 7, 8, 0, 10, 11, 0, 13, 14]
# slot layout: slots 0-4 = root(wrist) copies, slot(3f+l) = 5l+f
JOINT_OF_SLOT = {4: 0}
for _f in range(5):
    for _l in range(1, 4):
        JOINT_OF_SLOT[5 * _l + _f] = 3 * _f + _l


def _install_ntff_hook():
    """Register the axon NTFF profile hook if the image lacks antenv.axon_hooks."""
    try:
        import antenv
        if "antenv.axon_hooks" in sys.modules:
            return
        m = types.ModuleType("antenv.axon_hooks")
        m._hook = None
        m.set_axon_ntff_profile_hook = lambda h, _m=m: setattr(_m, "_hook", h)
        m.get_axon_ntff_profile_hook = lambda _m=m: _m._hook
        sys.modules["antenv.axon_hooks"] = m
        antenv.axon_hooks = m
        from trn_agent_boot.trn_boot import _ntff_profile_via_ctypes
        m.set_axon_ntff_profile_hook(
            _ntff_profile_via_ctypes("/opt/axon/libaxon_pjrt.so"))
    except Exception:
        pass


def host_constants(v_template, J_regressor, hands_comp, hands_mean, weights):
    """Batch-independent constants, computed in float64 then cast."""
    f64 = np.float64
    vt, Jr, hc, hm, W = (np.asarray(x, f64) for x in
                         (v_template, J_regressor, hands_comp, hands_mean, weights))
    Jt = (vt.T @ Jr).T                     # [21, 3] rest joints
    # pose matmul const: rows [theta(45), 1, wrist(3)] -> cols 3*s+c
    R = np.zeros((49, 60), f64)
    for s in range(20):
        if s < 5:
            R[46:49, 3 * s:3 * s + 3] = np.eye(3)
        else:
            j = JOINT_OF_SLOT[s]
            R[0:45, 3 * s:3 * s + 3] = hc[:, 3 * (j - 1):3 * (j - 1) + 3]
            R[45, 3 * s:3 * s + 3] = hm[3 * (j - 1):3 * (j - 1) + 3]
    # M' in slot order (slots 4..19)
    vh = np.concatenate([vt, np.ones((NV, 1))], axis=1)  # [NV, 4]
    Mp = np.zeros((NV, 16, 4), f64)
    for si in range(16):
        j = JOINT_OF_SLOT[4 + si]
        Mp[:, si, :] = W[:, j:j + 1] * vh
    JMp = np.einsum('vj,vsq->jsq', Jr, Mp)  # [21, 16, 4]
    Mf, JMf = Mp.reshape(NV, 64), JMp.reshape(NJOUT, 64)
    rhsD = np.zeros((64, NVJP), f64)
    rhsD[:, 0:NV] = Mf.T
    rhsD[:, NV:NV + NJOUT] = JMf.T
    # chain consts: layout [k, slot*4+cb], replicated over 128 partitions
    CT = np.zeros((20, 4), f64)   # local translations (k<3); root slots = Jt[0]
    CJ = np.zeros((16, 4), f64)   # rest joints for slots 4..19
    for s in range(5, 20):
        j = JOINT_OF_SLOT[s]
        CT[s, 0:3] = Jt[j] - Jt[PARENTS[j]]
    for s in range(5):
        CT[s, 0:3] = Jt[0]
    for si in range(16):
        CJ[si, 0:3] = Jt[JOINT_OF_SLOT[4 + si]]
    f32 = np.float32
    CTb = np.broadcast_to(CT.T[None, :, :, None], (128, 4, 20, 4))   # [p,k,s,cb]
    CJb = np.broadcast_to(CJ.T[None, :, :, None], (128, 4, 16, 4))
    return (np.ascontiguousarray(R, f32),
            np.ascontiguousarray(rhsD, f32),
            np.ascontiguousarray(CTb.reshape(128, 320), f32),
            np.ascontiguousarray(CJb.reshape(128, 256), f32))


def build_nc():
    nc = bacc.Bacc("TRN2", target_bir_lowering=False, debug=False,
                   num_devices=NCORES)
    theta_d = nc.dram_tensor("theta", [BC, 45], F32, kind="ExternalInput")
    wrist_d = nc.dram_tensor("wrist", [BC, 3], F32, kind="ExternalInput")
    r49_d = nc.dram_tensor("r49", [49, 60], F32, kind="ExternalInput")
    rhsd_d = nc.dram_tensor("rhsd", [64, NVJP], F32R, kind="ExternalInput")
    ct_d = nc.dram_tensor("ct", [128, 320], F32, kind="ExternalInput")
    cj_d = nc.dram_tensor("cj", [128, 256], F32, kind="ExternalInput")
    verts_d = nc.dram_tensor("verts", [BC, 3 * NV], F32, kind="ExternalOutput")
    joints_d = nc.dram_tensor("joints", [BC, 3 * NJOUT], F32, kind="ExternalOutput")

    with tile.TileContext(nc) as tc:
        with (
            tc.tile_pool(name="const", bufs=1) as cpool,
            tc.tile_pool(name="chain", bufs=1) as chp,
        ):
            # ---- constants into SBUF
            r49 = cpool.tile([49, 60], F32)
            nc.sync.dma_start(r49[:], r49_d[:])
            rhsd = cpool.tile([64, NVJP], F32R)
            nc.sync.dma_start(rhsd[:], rhsd_d[:])
            ct = cpool.tile([128, 4, 80], F32)       # [p, k, s*4+cb]
            nc.sync.dma_start(ct[:], ct_d[:].rearrange("p (k s) -> p k s", k=4))
            cj = cpool.tile([128, 4, 64], F32)       # [p, q, (s-4)*4+cb]
            nc.sync.dma_start(cj[:], cj_d[:].rearrange("p (k s) -> p k s", k=4))
            ident = cpool.tile([128, 128], F32)
            masks.make_identity(nc, ident[:])
            epsb = cpool.tile([128, 1], F32)
            nc.vector.memset(epsb[:], 1e-8)

            # ---- chain state tiles; sc = slot*4 + cb (80 wide)
            RAW = chp.tile([128, 3, 80], F32)    # [p, d, sc]
            AX = chp.tile([128, 3, 80], F32)
            SK = chp.tile([128, 3, 80], F32)
            SQ = chp.tile([128, 3, 80], F32)
            AA = chp.tile([128, 3, 3, 80], F32)
            S2 = chp.tile([128, 80], F32)
            ANG = chp.tile([128, 80], F32)
            INV = chp.tile([128, 80], F32)
            SIN = chp.tile([128, 80], F32)
            SH = chp.tile([128, 80], F32)
            CC = chp.tile([128, 80], F32)
            OC = chp.tile([128, 80], F32)
            RL = chp.tile([128, 12, 80], F32)    # [p, (r q), sc]
            AW = chp.tile([128, 12, 80], F32)
            TMPR = chp.tile([128, 3, 3, 20], F32)
            TMPT = chp.tile([128, 3, 1, 20], F32)
            ACC = chp.tile([128, 3, 1, 64], F32)
            TMPA = chp.tile([128, 3, 1, 64], F32)

            # ---- phase A: pose matmul per cb
            with (
                tc.tile_pool(name="posein", bufs=1) as pin_p,
                tc.tile_pool(name="posel", bufs=2) as pl_p,
                tc.tile_pool(name="posetr", bufs=1, space="PSUM") as ptr_p,
                tc.tile_pool(name="poseout", bufs=2, space="PSUM") as pout_p,
            ):
                # single DMA per input, then one transpose each
                th = pin_p.tile([128, NCB, 45], F32, tag="th")
                nc.sync.dma_start(
                    th[:], theta_d[:].rearrange("(c p) k -> p c k", p=128))
                wr = pin_p.tile([128, NCB, 3], F32, tag="wr")
                nc.sync.dma_start(
                    wr[:], wrist_d[:].rearrange("(c p) k -> p c k", p=128))
                tht = [ptr_p.tile([90, 128], F32, tag=f"tht{h}") for h in (0, 1)]
                for h in (0, 1):
                    nc.tensor.transpose(
                        tht[h][:],
                        th[:, 2 * h:2 * h + 2, :].rearrange("p c k -> p (c k)"),
                        ident[:])
                wrt = ptr_p.tile([NCB * 3, 128], F32, tag="wrt")
                nc.tensor.transpose(
                    wrt[:], wr[:].rearrange("p c k -> p (c k)"), ident[:])
                for cb in range(NCB):
                    pl = pl_p.tile([49, 128], F32)
                    o = 45 * (cb % 2)
                    nc.any.tensor_copy(pl[0:45, :], tht[cb // 2][o:o + 45, :])
                    nc.vector.memset(pl[45:46, :], 1.0)
                    nc.any.tensor_copy(pl[46:49, :], wrt[3 * cb:3 * cb + 3, :])
                    pps = pout_p.tile([128, 60], F32)
                    nc.tensor.matmul(pps[:], pl[:], r49[:], start=True, stop=True)
                    # scatter euler comps: RAW[:, d, s*4+cb] <- pps[:, 3s+d]
                    raw_sc = RAW[:].rearrange("p d (s c) -> p d s c", s=20)
                    for d in range(3):
                        src = pps[:].rearrange("p (s d) -> p d s", d=3)[:, d:d + 1, :]
                        nc.any.tensor_copy(
                            raw_sc[:, d:d + 1, :, cb:cb + 1].squeeze(3), src)

            # ---- PE warmup: keep TensorE busy through the chain phase so the
            # HAM clock gate is released before the LBS matmul burst.
            wlh = cpool.tile([64, 128], F32R, tag="wlh")
            nc.gpsimd.memset(wlh[:], 0.0)
            with tc.tile_pool(name="warm", bufs=1, space="PSUM") as wps_p:
                wps = wps_p.tile([128, 256], F32)
                for _ in range(N_WARM):
                    nc.tensor.matmul(wps[:], wlh[:], rhsd[:, 0:256],
                                     start=True, stop=True)

            # ---- phase B: rodrigues, all slots at once
            nc.any.tensor_tensor(SQ[:], RAW[:], RAW[:], op=ALU.mult)
            nc.any.tensor_tensor(S2[:], SQ[:, 0:1].squeeze(1), SQ[:, 1:2].squeeze(1),
                                 op=ALU.add)
            nc.any.tensor_tensor(S2[:], S2[:], SQ[:, 2:3].squeeze(1), op=ALU.add)
            nc.scalar.activation(ANG[:], S2[:], AF.Sqrt, bias=epsb[:])
            nc.vector.reciprocal(INV[:], ANG[:])
            nc.scalar.activation(SIN[:], ANG[:], AF.Sin)
            nc.scalar.activation(SH[:], ANG[:], AF.Sin, scale=0.5)
            nc.any.tensor_tensor(CC[:], SH[:], SH[:], op=ALU.mult)
            nc.any.tensor_scalar(CC[:], CC[:], -2.0, 1.0, op0=ALU.mult, op1=ALU.add)
            nc.any.tensor_scalar(OC[:], CC[:], -1.0, 1.0, op0=ALU.mult, op1=ALU.add)
            invb = INV[:].unsqueeze(1).broadcast_to([128, 3, 80])
            nc.any.tensor_tensor(AX[:], RAW[:], invb, op=ALU.mult)
            sinb = SIN[:].unsqueeze(1).broadcast_to([128, 3, 80])
            nc.any.tensor_tensor(SK[:], AX[:], sinb, op=ALU.mult)
            a_r = AX[:].unsqueeze(2).broadcast_to([128, 3, 3, 80])
            a_c = AX[:].unsqueeze(1).broadcast_to([128, 3, 3, 80])
            nc.any.tensor_tensor(AA[:], a_r, a_c, op=ALU.mult)
            ocb = OC[:].unsqueeze(1).unsqueeze(1).broadcast_to([128, 3, 3, 80])
            nc.any.tensor_tensor(AA[:], AA[:], ocb, op=ALU.mult)
            # write R entries into RL (e = r*4+c)
            rl_e = RL[:].rearrange("p (r q) sc -> p r q sc", r=3)
            for r in range(3):
                for c in range(3):
                    dst = rl_e[:, r:r + 1, c:c + 1, :]
                    aslc = AA[:, r:r + 1, c:c + 1, :]
                    if r == c:
                        nc.any.tensor_tensor(dst, aslc,
                                             CC[:].unsqueeze(1).unsqueeze(1),
                                             op=ALU.add)
                    else:
                        k = 3 - r - c
                        pos = (r, c) in ((1, 0), (0, 2), (2, 1))
                        nc.any.tensor_tensor(
                            dst, aslc,
                            SK[:, k:k + 1, :].unsqueeze(1),
                            op=ALU.add if pos else ALU.subtract)

            # ---- phase C: root init (slots 0..4 -> sc 0..20)
            aw_e = AW[:].rearrange("p (r q) sc -> p r q sc", r=3)
            nc.any.tensor_copy(aw_e[:, :, 0:3, 0:20], rl_e[:, :, 0:3, 0:20])
            nc.any.tensor_copy(aw_e[:, :, 3:4, 0:20],
                               ct[:, 0:3, 0:20].unsqueeze(2))

            # ---- phase D: kinematic levels
            for lvl in (1, 2, 3):
                Cs, Ce = 20 * lvl, 20 * lvl + 20
                Ps, Pe = 20 * (lvl - 1), 20 * lvl
                # rotation: AW[C](r,c) = sum_k AW[P](r,k) * RL[C](k,c)
                for k in range(3):
                    in0 = aw_e[:, :, k:k + 1, Ps:Pe].broadcast_to([128, 3, 3, 20])
                    in1 = rl_e[:, k:k + 1, 0:3, Cs:Ce].broadcast_to([128, 3, 3, 20])
                    if k == 0:
                        nc.any.tensor_tensor(aw_e[:, :, 0:3, Cs:Ce], in0, in1,
                                             op=ALU.mult)
                    else:
                        nc.any.tensor_tensor(TMPR[:], in0, in1, op=ALU.mult)
                        nc.any.tensor_tensor(aw_e[:, :, 0:3, Cs:Ce],
                                             aw_e[:, :, 0:3, Cs:Ce], TMPR[:],
                                             op=ALU.add)
                # translation: AW[C](r,3) = sum_k AW[P](r,k)*CT[C][k] + AW[P](r,3)
                tdst = aw_e[:, :, 3:4, Cs:Ce]
                for k in range(3):
                    in0 = aw_e[:, :, k:k + 1, Ps:Pe]
                    ctk = ct[:, k:k + 1, Cs:Ce].unsqueeze(1).broadcast_to(
                        [128, 3, 1, 20])
                    if k == 0:
                        nc.any.tensor_tensor(tdst, in0, ctk, op=ALU.mult)
                    else:
                        nc.any.tensor_tensor(TMPT[:], in0, ctk, op=ALU.mult)
                        nc.any.tensor_tensor(tdst, tdst, TMPT[:], op=ALU.add)
                nc.any.tensor_tensor(tdst, tdst, aw_e[:, :, 3:4, Ps:Pe],
                                     op=ALU.add)

            # ---- A_rel: t -= R @ J  (slots 4..19 -> sc 16..80)
            for q in range(3):
                in0 = aw_e[:, :, q:q + 1, 16:80]
                cjq = cj[:, q:q + 1, :].unsqueeze(1).broadcast_to([128, 3, 1, 64])
                if q == 0:
                    nc.any.tensor_tensor(ACC[:], in0, cjq, op=ALU.mult)
                else:
                    nc.any.tensor_tensor(TMPA[:], in0, cjq, op=ALU.mult)
                    nc.any.tensor_tensor(ACC[:], ACC[:], TMPA[:], op=ALU.add)
            nc.any.tensor_tensor(aw_e[:, :, 3:4, 16:80],
                                 aw_e[:, :, 3:4, 16:80], ACC[:], op=ALU.subtract)

            # ---- phase E: lhsT build + LBS matmuls + output
            # aw view [p, r, q, s, cb]
            aw_pq = AW[:].rearrange("p (r q) (s c) -> p r q s c", r=3, s=20)
            with (
                tc.tile_pool(name="ltr", bufs=2, space="PSUM") as ltr_p,
                tc.tile_pool(name="lhs", bufs=2) as lhs_p,
                tc.tile_pool(name="lbs", bufs=4, space="PSUM") as lbs_p,
                tc.tile_pool(name="vout", bufs=2) as vout_p,
            ):
                for cb in range(NCB):
                    # per-coordinate lhsT_p [64, 128] = Aflat_p^T for this cb
                    lhs = []
                    for p in range(3):
                        g = lhs_p.tile([128, 64], F32, tag="g")
                        nc.any.tensor_copy(
                            g[:].rearrange("p (s q) -> p s q", s=16),
                            aw_pq[:, p:p + 1, :, 4:20, cb:cb + 1]
                            .squeeze(4).squeeze(1).transpose([0, 2, 1]))
                        tr = ltr_p.tile([64, 128], F32)
                        nc.tensor.transpose(tr[:], g[:], ident[:])
                        lh = lhs_p.tile([64, 128], F32R, tag=f"lhs{p}")
                        nc.any.tensor_copy(lh[:], tr[:])
                        lhs.append(lh)
                    out_sb = vout_p.tile([128, 3 * NVJP], F32)
                    out_i = out_sb[:].rearrange("p (v q) -> p v q", q=3)
                    for p in range(3):
                        for (v0, nn) in VCHUNKS:
                            ps = lbs_p.tile([128, 370], F32)
                            nc.tensor.matmul(ps[:, 0:nn], lhs[p][:],
                                             rhsd[:, v0:v0 + nn],
                                             start=True, stop=True)
                            nc.any.tensor_copy(
                                out_i[:, v0:v0 + nn, p:p + 1].squeeze(2),
                                ps[:, 0:nn])
                    r0, r1 = 128 * cb, 128 * (cb + 1)
                    nc.sync.dma_start(verts_d[r0:r1, :], out_sb[:, 0:3 * NV])
                    nc.sync.dma_start(joints_d[r0:r1, :],
                                      out_sb[:, 3 * NV:3 * NV + 3 * NJOUT])
    nc.finalize()
    return nc


_CACHE = {}


def _get_nc():
    if "nc" not in _CACHE:
        _CACHE["nc"] = build_nc()
    return _CACHE["nc"]


def kernel(beta, theta, wrist_euler, v_template, J_regressor, hands_comp,
           hands_mean, weights):
    _install_ntff_hook()
    f32 = np.float32
    theta = np.ascontiguousarray(np.asarray(theta), f32)
    wrist = np.ascontiguousarray(np.asarray(wrist_euler), f32)
    r49, rhsd, ct, cj = host_constants(
        v_template, J_regressor, hands_comp, hands_mean, weights)
    nc = _get_nc()
    in_maps = []
    for c in range(NCORES):
        sl = slice(c * BC, (c + 1) * BC)
        in_maps.append({
            "theta": theta[sl], "wrist": wrist[sl],
            "r49": r49, "rhsd": rhsd, "ct": ct, "cj": cj,
        })
    res = run_bass_kernel_spmd(nc, in_maps, core_ids=list(range(NCORES)))
    verts = np.concatenate([res.results[c]["verts"] for c in range(NCORES)], axis=0)
    joints = np.concatenate([res.results[c]["joints"] for c in range(NCORES)], axis=0)
    return verts.reshape(B, NV, 3), joints.reshape(B, NJOUT, 3)
